# revision 14
# baseline (speedup 1.0000x reference)
"""DescriptorMatcher (SNN ratio-test) Trainium2 kernel.

Problem: desc1, desc2 (8192, 128) f32. For each row i of desc1: find the two
nearest neighbours in desc2 (L2), ratio = d1/d2, mask = ratio <= 0.8, emit
(ratio-or-0, [i, argmin]-or--1, mask).

Sharding: desc1 rows split across 8 cores (1024 rows each), desc2 replicated.
Host-side prep is layout/encoding only: slicing, transposes, and a bf16
hi/lo two-term encoding of the descriptors (hi = bf16(x), lo = bf16(x-hi));
all O(B1*B2) math runs on device.

Per-core algorithm:
  - Ranking surface s = d1.d2 - ||d2||^2/2 = -(dist^2 - ||d1||^2)/2 per
    (128 x 512) chunk: 3 bf16 matmul passes (hi*hi + hi*lo + lo*hi) plus a
    K=2 matmul folding -||d2||^2/2 (bf16 hi/lo rows, computed on device).
  - ScalarE copies PSUM (128 x 1024 double-chunks) into SBUF half-row tiles.
  - Per half (128 x 4096): VectorE max8 + find_index8.
  - Merge halves with a compare/select chain -> global j1, j2.
  - Exact refinement: indirect-DMA gather desc2[j1], desc2[j2]; recompute
    both squared distances exactly in fp32; ratio = sqrt(d1sq/d2sq)
    (ACT sqrt + one Newton step); mask = ratio <= 0.8; masked outputs.
Outputs per core: ratio (128,8) f32, idx (128,8) i32, mask (128,8) u8;
column rt = row-tile rt (rows rt*128 .. rt*128+127 of the core's slice).
"""

import numpy as np
import ml_dtypes

import concourse.bass as bass
import concourse.mybir as mybir
from concourse.bacc import Bacc
from concourse.tile import TileContext
from concourse.bass_utils import run_bass_kernel_spmd

P = 128
B1 = 8192
B2 = 8192
D = 128
NCORES = 8
ROWS_PER_CORE = B1 // NCORES       # 1024
RT = ROWS_PER_CORE // P            # 8 row-tiles per core
N = 512                            # matmul free-dim chunk
NCH = B2 // N                      # 16 chunks
HALF = B2 // 2                     # 4096
CH_H = NCH // 2                    # 8 chunks per half
TH = 0.8

F32 = mybir.dt.float32
BF16 = mybir.dt.bfloat16
I32 = mybir.dt.int32
U8 = mybir.dt.uint8
U32 = mybir.dt.uint32
AF = mybir.ActivationFunctionType
OP = mybir.AluOpType

_CACHE = {}


def build_nc():
    nc = Bacc()
    d1n = nc.dram_tensor("d1n", [ROWS_PER_CORE, D], F32, kind="ExternalInput")
    d1h = nc.dram_tensor("d1h", [D, ROWS_PER_CORE], BF16, kind="ExternalInput")
    d1l = nc.dram_tensor("d1l", [D, ROWS_PER_CORE], BF16, kind="ExternalInput")
    d2t = nc.dram_tensor("d2t", [D, B2], F32, kind="ExternalInput")
    d2h = nc.dram_tensor("d2h", [D, B2], BF16, kind="ExternalInput")
    d2l = nc.dram_tensor("d2l", [D, B2], BF16, kind="ExternalInput")
    d2n = nc.dram_tensor("d2n", [B2, D], F32, kind="ExternalInput")
    o_ratio = nc.dram_tensor("o_ratio", [P, RT], F32, kind="ExternalOutput")
    o_idx = nc.dram_tensor("o_idx", [P, RT], I32, kind="ExternalOutput")
    o_mask = nc.dram_tensor("o_mask", [P, RT], U8, kind="ExternalOutput")

    with TileContext(nc) as tc:
        with tc.tile_pool(name="const", bufs=1) as cpool, \
             tc.tile_pool(name="load", bufs=3) as ldpool, \
             tc.tile_pool(name="setup", bufs=2) as supool, \
             tc.tile_pool(name="scan", bufs=4) as spool, \
             tc.tile_pool(name="small", bufs=3) as smpool, \
             tc.tile_pool(name="ps_set", bufs=2, space="PSUM") as psB, \
             tc.tile_pool(name="ps_mm", bufs=3, space="PSUM") as psA:

            # ---------------- persistent tiles ----------------
            d2hi = cpool.tile([P, B2], BF16, tag="d2hi")
            d2lo = cpool.tile([P, B2], BF16, tag="d2lo")
            cf = cpool.tile([2, B2], BF16, tag="cf")       # -d2sq/2: hi, lo rows
            d1hi = cpool.tile([P, ROWS_PER_CORE], BF16, tag="d1hi")
            d1lo = cpool.tile([P, ROWS_PER_CORE], BF16, tag="d1lo")
            mhalf = cpool.tile([P, P], F32, tag="mhalf")   # -0.5 weights
            ones2 = cpool.tile([2, P], BF16, tag="ones2")
            outr = cpool.tile([P, RT], F32, tag="outr")
            outi = cpool.tile([P, RT], F32, tag="outi")
            outm = cpool.tile([P, RT], F32, tag="outm")

            nc.gpsimd.memset(mhalf, -0.5)
            nc.gpsimd.memset(ones2, 1.0)
            nc.sync.dma_start(out=d1hi, in_=d1h[:, :])
            nc.sync.dma_start(out=d1lo, in_=d1l[:, :])

            # ---------------- setup: c rows + d2 split loads, per chunk ----
            for n in range(NCH):
                sl = bass.ts(n, N)
                nc.sync.dma_start(out=d2hi[:, sl], in_=d2h[:, sl])
                nc.sync.dma_start(out=d2lo[:, sl], in_=d2l[:, sl])
                d2c = ldpool.tile([P, N], F32, tag="d2c")
                nc.sync.dma_start(out=d2c, in_=d2t[:, sl])
                sq = supool.tile([P, N], F32, tag="sq")
                nc.scalar.activation(sq, d2c, AF.Square)
                cps = psB.tile([P, N], F32, tag="csq")
                nc.tensor.matmul(cps, mhalf, sq, start=True, stop=True)
                chi = supool.tile([P, N], BF16, tag="chi")
                nc.scalar.copy(chi, cps)
                crem = supool.tile([P, N], F32, tag="crem")
                nc.vector.tensor_sub(crem, cps, chi)             # DVE, PSUM src
                clo = supool.tile([P, N], BF16, tag="clo")
                nc.scalar.copy(clo, crem)
                nc.sync.dma_start(out=cf[0:1, sl], in_=chi[0:1, :])
                nc.sync.dma_start(out=cf[1:2, sl], in_=clo[0:1, :])

            # ---------------- steady: per row-tile --------------------------
            for rt in range(RT):
                rsl = bass.ts(rt, P)
                l_hi = d1hi[:, rsl]
                l_lo = d1lo[:, rsl]
                d1nc = ldpool.tile([P, D], F32, tag="d1nc")
                nc.sync.dma_start(out=d1nc, in_=d1n[rsl, :])

                CV = []   # per half: (128,8) top values
                IV = []   # per half: (128,8) u32 indices (half-local)
                for h in range(2):
                    s_half = spool.tile([P, HALF], F32, tag="s")
                    for kk in range(CH_H // 2):      # double-chunks of 1024
                        ps = psA.tile([P, 2 * N], F32, tag="mm")
                        for j in range(2):
                            n = h * CH_H + kk * 2 + j
                            sl = bass.ts(n, N)
                            pslice = ps[:, bass.ts(j, N)]
                            nc.tensor.matmul(pslice, l_hi, d2hi[:, sl], start=True, stop=False)
                            nc.tensor.matmul(pslice, l_hi, d2lo[:, sl], start=False, stop=False)
                            nc.tensor.matmul(pslice, l_lo, d2hi[:, sl], start=False, stop=False)
                            nc.tensor.matmul(pslice, ones2, cf[:, sl], start=False, stop=True)
                        nc.scalar.copy(s_half[:, bass.ds(kk * 2 * N, N)], ps[:, 0:N])
                        nc.scalar.copy(s_half[:, bass.ds(kk * 2 * N + N, N)], ps[:, N:2 * N])
                    m8 = smpool.tile([P, 8], F32, tag="m8")
                    nc.vector.max(out=m8, in_=s_half)
                    i8 = smpool.tile([P, 8], U32, tag="i8")
                    nc.vector.max_index(out=i8, in_max=m8, in_values=s_half)
                    CV.append(m8)
                    IV.append(i8)

                # ---- merge halves: global (j1, j2) via select chain --------
                A0 = CV[0][:, 0:1]; A1 = CV[0][:, 1:2]
                B0 = CV[1][:, 0:1]; B1 = CV[1][:, 1:2]
                IA0 = smpool.tile([P, 1], F32, tag="IA0")
                nc.gpsimd.tensor_copy(IA0, IV[0][:, 0:1])
                IA1 = smpool.tile([P, 1], F32, tag="IA1")
                nc.gpsimd.tensor_copy(IA1, IV[0][:, 1:2])
                IB0 = smpool.tile([P, 1], F32, tag="IB0")
                nc.gpsimd.tensor_copy(IB0, IV[1][:, 0:1])
                nc.gpsimd.tensor_scalar_add(IB0, IB0, float(HALF))
                IB1 = smpool.tile([P, 1], F32, tag="IB1")
                nc.gpsimd.tensor_copy(IB1, IV[1][:, 1:2])
                nc.gpsimd.tensor_scalar_add(IB1, IB1, float(HALF))

                c0 = smpool.tile([P, 1], U8, tag="c0")   # 1 if half0 wins
                nc.vector.tensor_tensor(c0, A0, B0, op=OP.is_ge)
                c1 = smpool.tile([P, 1], U8, tag="c1")   # A1 vs B0
                nc.vector.tensor_tensor(c1, A1, B0, op=OP.is_ge)
                c2 = smpool.tile([P, 1], U8, tag="c2")   # A0 vs B1
                nc.vector.tensor_tensor(c2, A0, B1, op=OP.is_ge)

                # j1 = c0 ? IA0 : IB0
                j1 = smpool.tile([P, 1], F32, tag="j1")
                nc.vector.select(j1, c0, IA0, IB0)
                # j2 = c0 ? (c1 ? IA1 : IB0) : (c2 ? IA0 : IB1)
                j2a = smpool.tile([P, 1], F32, tag="j2a")
                nc.vector.select(j2a, c1, IA1, IB0)
                j2b = smpool.tile([P, 1], F32, tag="j2b")
                nc.vector.select(j2b, c2, IA0, IB1)
                j2 = smpool.tile([P, 1], F32, tag="j2")
                nc.vector.select(j2, c0, j2a, j2b)

                j1u = smpool.tile([P, 1], U32, tag="j1u")
                nc.gpsimd.tensor_copy(j1u, j1)
                j2u = smpool.tile([P, 1], U32, tag="j2u")
                nc.gpsimd.tensor_copy(j2u, j2)

                # ---- exact refinement ----
                g1 = smpool.tile([P, D], F32, tag="g1")
                nc.gpsimd.indirect_dma_start(
                    out=g1, out_offset=None, in_=d2n[:, :],
                    in_offset=bass.IndirectOffsetOnAxis(ap=j1u[:, 0:1], axis=0))
                g2 = smpool.tile([P, D], F32, tag="g2")
                nc.gpsimd.indirect_dma_start(
                    out=g2, out_offset=None, in_=d2n[:, :],
                    in_offset=bass.IndirectOffsetOnAxis(ap=j2u[:, 0:1], axis=0))

                t1 = smpool.tile([P, D], F32, tag="t1")
                nc.gpsimd.tensor_sub(t1, g1, d1nc)
                t2 = smpool.tile([P, D], F32, tag="t2")
                nc.gpsimd.tensor_sub(t2, g2, d1nc)
                sc1 = smpool.tile([P, D], F32, tag="sc1")
                nc.gpsimd.tensor_mul(sc1, t1, t1)
                dist1 = smpool.tile([P, 1], F32, tag="dist1")
                nc.vector.reduce_sum(dist1, sc1, axis=mybir.AxisListType.X)
                sc2 = smpool.tile([P, D], F32, tag="sc2")
                nc.gpsimd.tensor_mul(sc2, t2, t2)
                dist2 = smpool.tile([P, 1], F32, tag="dist2")
                nc.vector.reduce_sum(dist2, sc2, axis=mybir.AxisListType.X)

                # ratio = sqrt(dist1/dist2), Newton-refined
                rec = smpool.tile([P, 1], F32, tag="rec")
                nc.vector.reciprocal(rec, dist2)
                q = smpool.tile([P, 1], F32, tag="q")
                nc.gpsimd.tensor_mul(q, dist1, rec)
                r0 = smpool.tile([P, 1], F32, tag="r0")
                nc.scalar.activation(r0, q, AF.Sqrt)
                rr = smpool.tile([P, 1], F32, tag="rr")
                nc.vector.reciprocal(rr, r0)
                t = smpool.tile([P, 1], F32, tag="t")
                nc.gpsimd.tensor_mul(t, q, rr)
                r1 = smpool.tile([P, 1], F32, tag="r1")
                nc.gpsimd.tensor_add(r1, r0, t)
                nc.gpsimd.tensor_scalar_mul(r1, r1, 0.5)

                maskf = smpool.tile([P, 1], F32, tag="maskf")
                nc.vector.tensor_scalar(maskf, r1, TH, scalar2=None, op0=OP.is_le)
                nc.gpsimd.tensor_mul(outr[:, rt:rt + 1], r1, maskf)
                nc.gpsimd.tensor_copy(outm[:, rt:rt + 1], maskf)

                jm = smpool.tile([P, 1], F32, tag="jm")
                nc.gpsimd.tensor_scalar_add(jm, j1, 1.0)
                nc.gpsimd.tensor_mul(jm, jm, maskf)
                nc.gpsimd.tensor_scalar_sub(outi[:, rt:rt + 1], jm, 1.0)

            # ---------------- outputs ---------------------------------------
            outi_i = cpool.tile([P, RT], I32, tag="outi_i")
            nc.gpsimd.tensor_copy(outi_i, outi)
            outm_u = cpool.tile([P, RT], U8, tag="outm_u")
            nc.gpsimd.tensor_copy(outm_u, outm)
            nc.sync.dma_start(out=o_ratio[:, :], in_=outr)
            nc.sync.dma_start(out=o_idx[:, :], in_=outi_i)
            nc.sync.dma_start(out=o_mask[:, :], in_=outm_u)

    nc.finalize()
    return nc


def _get_nc():
    if "nc" not in _CACHE:
        _CACHE["nc"] = build_nc()
    return _CACHE["nc"]


def _hilo(x):
    """bf16 two-term encoding (hi = bf16(x), lo = bf16(x - hi))."""
    hi = x.astype(ml_dtypes.bfloat16)
    lo = (x - hi.astype(np.float32)).astype(ml_dtypes.bfloat16)
    return hi, lo


def kernel(desc1, desc2, _trace=False, _tmpdir=None):
    desc1 = np.ascontiguousarray(np.asarray(desc1, dtype=np.float32))
    desc2 = np.ascontiguousarray(np.asarray(desc2, dtype=np.float32))
    assert desc1.shape == (B1, D) and desc2.shape == (B2, D)

    d2t = np.ascontiguousarray(desc2.T)
    d2hv, d2lv = _hilo(d2t)
    d2hv = np.ascontiguousarray(d2hv)
    d2lv = np.ascontiguousarray(d2lv)
    in_maps = []
    for c in range(NCORES):
        sl = slice(c * ROWS_PER_CORE, (c + 1) * ROWS_PER_CORE)
        d1n_c = np.ascontiguousarray(desc1[sl])
        d1t_c = np.ascontiguousarray(d1n_c.T)
        d1h_c, d1l_c = _hilo(d1t_c)
        in_maps.append({
            "d1n": d1n_c,
            "d1h": np.ascontiguousarray(d1h_c),
            "d1l": np.ascontiguousarray(d1l_c),
            "d2t": d2t, "d2h": d2hv, "d2l": d2lv, "d2n": desc2,
        })

    nc = _get_nc()
    res = run_bass_kernel_spmd(
        nc, in_maps, core_ids=list(range(NCORES)),
        trace=_trace, tmpdir=_tmpdir,
    )
    if _trace:
        _CACHE["last_result"] = res

    ratios, idxs, masks = [], [], []
    for c in range(NCORES):
        r = res.results[c]
        ratios.append(r["o_ratio"].T.reshape(-1))
        idxs.append(r["o_idx"].T.reshape(-1))
        masks.append(r["o_mask"].T.reshape(-1))
    ratio = np.concatenate(ratios).astype(np.float32)
    idx1 = np.concatenate(idxs).astype(np.int32)
    mask = np.concatenate(masks).astype(bool)

    match_dists = ratio[:, None]
    col0 = np.where(mask, np.arange(B1, dtype=np.int32), np.int32(-1))
    matches_idxs = np.stack([col0, idx1], axis=1).astype(np.int32)
    return match_dists, matches_idxs, mask


# revision 15
# speedup vs baseline: 1.1036x; 1.1036x over previous
"""DescriptorMatcher (SNN ratio-test) Trainium2 kernel.

Problem: desc1, desc2 (8192, 128) f32. For each row i of desc1: find the two
nearest neighbours in desc2 (L2), ratio = d1/d2, mask = ratio <= 0.8, emit
(ratio-or-0, [i, argmin]-or--1, mask).

Sharding: desc1 rows split across 8 cores (1024 rows each), desc2 replicated.
Host-side prep is layout/encoding only: slicing, transposes, and a bf16
hi/lo two-term encoding of the descriptors (hi = bf16(x), lo = bf16(x-hi));
all O(B1*B2) math runs on device.

Per-core algorithm:
  - Ranking surface s = d1.d2 - ||d2||^2/2 = -(dist^2 - ||d1||^2)/2 per
    (128 x 512) chunk: 3 bf16 matmul passes (hi*hi + hi*lo + lo*hi) plus a
    K=2 matmul folding -||d2||^2/2 (bf16 hi/lo rows, computed on device).
  - ScalarE copies PSUM (128 x 1024 double-chunks) into SBUF half-row tiles.
  - Per half (128 x 4096): VectorE max8 + find_index8.
  - Merge halves with a compare/select chain -> global j1, j2.
  - Exact refinement: indirect-DMA gather desc2[j1], desc2[j2]; recompute
    both squared distances exactly in fp32; ratio = sqrt(d1sq/d2sq)
    (ACT sqrt + one Newton step); mask = ratio <= 0.8; masked outputs.
Outputs per core: ratio (128,8) f32, idx (128,8) i32, mask (128,8) u8;
column rt = row-tile rt (rows rt*128 .. rt*128+127 of the core's slice).
"""

import numpy as np
import ml_dtypes

import concourse.bass as bass
import concourse.mybir as mybir
from concourse.bacc import Bacc
from concourse.tile import TileContext
from concourse.bass_utils import run_bass_kernel_spmd

P = 128
B1 = 8192
B2 = 8192
D = 128
NCORES = 8
ROWS_PER_CORE = B1 // NCORES       # 1024
RT = ROWS_PER_CORE // P            # 8 row-tiles per core
N = 512                            # matmul free-dim chunk
NCH = B2 // N                      # 16 chunks
HALF = B2 // 2                     # 4096
CH_H = NCH // 2                    # 8 chunks per half
TH = 0.8

F32 = mybir.dt.float32
BF16 = mybir.dt.bfloat16
I32 = mybir.dt.int32
U8 = mybir.dt.uint8
U32 = mybir.dt.uint32
AF = mybir.ActivationFunctionType
OP = mybir.AluOpType

_CACHE = {}


def build_nc():
    nc = Bacc()
    d1n = nc.dram_tensor("d1n", [ROWS_PER_CORE, D], F32, kind="ExternalInput")
    d1h = nc.dram_tensor("d1h", [D, ROWS_PER_CORE], BF16, kind="ExternalInput")
    d1l = nc.dram_tensor("d1l", [D, ROWS_PER_CORE], BF16, kind="ExternalInput")
    d2t = nc.dram_tensor("d2t", [D, B2], F32, kind="ExternalInput")
    d2h = nc.dram_tensor("d2h", [D, B2], BF16, kind="ExternalInput")
    d2l = nc.dram_tensor("d2l", [D, B2], BF16, kind="ExternalInput")
    d2n = nc.dram_tensor("d2n", [B2, D], F32, kind="ExternalInput")
    o_ratio = nc.dram_tensor("o_ratio", [P, RT], F32, kind="ExternalOutput")
    o_idx = nc.dram_tensor("o_idx", [P, RT], I32, kind="ExternalOutput")
    o_mask = nc.dram_tensor("o_mask", [P, RT], U8, kind="ExternalOutput")

    with TileContext(nc) as tc:
        with tc.tile_pool(name="const", bufs=1) as cpool, \
             tc.tile_pool(name="load", bufs=3) as ldpool, \
             tc.tile_pool(name="setup", bufs=2) as supool, \
             tc.tile_pool(name="scan", bufs=4) as spool, \
             tc.tile_pool(name="small", bufs=3) as smpool, \
             tc.tile_pool(name="ps_set", bufs=2, space="PSUM") as psB, \
             tc.tile_pool(name="ps_mm", bufs=5, space="PSUM") as psA:

            # ---------------- persistent tiles ----------------
            d2hi = cpool.tile([P, B2], BF16, tag="d2hi")
            d2lo = cpool.tile([P, B2], BF16, tag="d2lo")
            cf = cpool.tile([2, B2], BF16, tag="cf")       # -d2sq/2: hi, lo rows
            d1hi = cpool.tile([P, ROWS_PER_CORE], BF16, tag="d1hi")
            d1lo = cpool.tile([P, ROWS_PER_CORE], BF16, tag="d1lo")
            mhalf = cpool.tile([P, P], F32, tag="mhalf")   # -0.5 weights
            ones2 = cpool.tile([2, P], BF16, tag="ones2")
            outr = cpool.tile([P, RT], F32, tag="outr")
            outi = cpool.tile([P, RT], F32, tag="outi")
            outm = cpool.tile([P, RT], F32, tag="outm")

            nc.gpsimd.memset(mhalf, -0.5)
            nc.gpsimd.memset(ones2, 1.0)
            nc.sync.dma_start(out=d1hi, in_=d1h[:, :])
            nc.sync.dma_start(out=d1lo, in_=d1l[:, :])

            # ---------------- setup: c rows + d2 split loads, per chunk ----
            for n in range(NCH):
                sl = bass.ts(n, N)
                nc.sync.dma_start(out=d2hi[:, sl], in_=d2h[:, sl])
                nc.sync.dma_start(out=d2lo[:, sl], in_=d2l[:, sl])
                d2c = ldpool.tile([P, N], F32, tag="d2c")
                nc.sync.dma_start(out=d2c, in_=d2t[:, sl])
                sq = supool.tile([P, N], F32, tag="sq")
                nc.scalar.activation(sq, d2c, AF.Square)
                cps = psB.tile([P, N], F32, tag="csq")
                nc.tensor.matmul(cps, mhalf, sq, start=True, stop=True)
                chi = supool.tile([P, N], BF16, tag="chi")
                nc.scalar.copy(chi, cps)
                crep = supool.tile([P, N], F32, tag="crep")
                nc.scalar.copy(crep, cps)
                crem = supool.tile([P, N], F32, tag="crem")
                nc.gpsimd.tensor_sub(crem, crep, chi)
                clo = supool.tile([P, N], BF16, tag="clo")
                nc.scalar.copy(clo, crem)
                nc.sync.dma_start(out=cf[0:1, sl], in_=chi[0:1, :])
                nc.sync.dma_start(out=cf[1:2, sl], in_=clo[0:1, :])

            # ---------------- steady: per row-tile --------------------------
            for rt in range(RT):
                rsl = bass.ts(rt, P)
                l_hi = d1hi[:, rsl]
                l_lo = d1lo[:, rsl]
                d1nc = ldpool.tile([P, D], F32, tag="d1nc")
                nc.sync.dma_start(out=d1nc, in_=d1n[rsl, :])

                CV = []   # per half: (128,8) top values
                IV = []   # per half: (128,8) u32 indices (half-local)
                for h in range(2):
                    s_half = spool.tile([P, HALF], F32, tag="s")
                    for k in range(CH_H):
                        n = h * CH_H + k
                        sl = bass.ts(n, N)
                        ps = psA.tile([P, N], F32, tag="mm")
                        nc.tensor.matmul(ps, l_hi, d2hi[:, sl], start=True, stop=False)
                        nc.tensor.matmul(ps, l_hi, d2lo[:, sl], start=False, stop=False)
                        nc.tensor.matmul(ps, l_lo, d2hi[:, sl], start=False, stop=False)
                        nc.tensor.matmul(ps, ones2, cf[:, sl], start=False, stop=True)
                        nc.scalar.copy(s_half[:, bass.ts(k, N)], ps)
                    m8 = smpool.tile([P, 8], F32, tag="m8")
                    nc.vector.max(out=m8, in_=s_half)
                    i8 = smpool.tile([P, 8], U32, tag="i8")
                    nc.vector.max_index(out=i8, in_max=m8, in_values=s_half)
                    CV.append(m8)
                    IV.append(i8)

                # ---- merge halves: global (j1, j2) via select chain --------
                A0 = CV[0][:, 0:1]; A1 = CV[0][:, 1:2]
                B0 = CV[1][:, 0:1]; B1 = CV[1][:, 1:2]
                IA0 = smpool.tile([P, 1], F32, tag="IA0")
                nc.gpsimd.tensor_copy(IA0, IV[0][:, 0:1])
                IA1 = smpool.tile([P, 1], F32, tag="IA1")
                nc.gpsimd.tensor_copy(IA1, IV[0][:, 1:2])
                IB0 = smpool.tile([P, 1], F32, tag="IB0")
                nc.gpsimd.tensor_copy(IB0, IV[1][:, 0:1])
                nc.gpsimd.tensor_scalar_add(IB0, IB0, float(HALF))
                IB1 = smpool.tile([P, 1], F32, tag="IB1")
                nc.gpsimd.tensor_copy(IB1, IV[1][:, 1:2])
                nc.gpsimd.tensor_scalar_add(IB1, IB1, float(HALF))

                c0 = smpool.tile([P, 1], U8, tag="c0")   # 1 if half0 wins
                nc.vector.tensor_tensor(c0, A0, B0, op=OP.is_ge)
                c1 = smpool.tile([P, 1], U8, tag="c1")   # A1 vs B0
                nc.vector.tensor_tensor(c1, A1, B0, op=OP.is_ge)
                c2 = smpool.tile([P, 1], U8, tag="c2")   # A0 vs B1
                nc.vector.tensor_tensor(c2, A0, B1, op=OP.is_ge)

                # j1 = c0 ? IA0 : IB0
                j1 = smpool.tile([P, 1], F32, tag="j1")
                nc.vector.select(j1, c0, IA0, IB0)
                # j2 = c0 ? (c1 ? IA1 : IB0) : (c2 ? IA0 : IB1)
                j2a = smpool.tile([P, 1], F32, tag="j2a")
                nc.vector.select(j2a, c1, IA1, IB0)
                j2b = smpool.tile([P, 1], F32, tag="j2b")
                nc.vector.select(j2b, c2, IA0, IB1)
                j2 = smpool.tile([P, 1], F32, tag="j2")
                nc.vector.select(j2, c0, j2a, j2b)

                j1u = smpool.tile([P, 1], U32, tag="j1u")
                nc.gpsimd.tensor_copy(j1u, j1)
                j2u = smpool.tile([P, 1], U32, tag="j2u")
                nc.gpsimd.tensor_copy(j2u, j2)

                # ---- exact refinement ----
                g1 = smpool.tile([P, D], F32, tag="g1")
                nc.gpsimd.indirect_dma_start(
                    out=g1, out_offset=None, in_=d2n[:, :],
                    in_offset=bass.IndirectOffsetOnAxis(ap=j1u[:, 0:1], axis=0))
                g2 = smpool.tile([P, D], F32, tag="g2")
                nc.gpsimd.indirect_dma_start(
                    out=g2, out_offset=None, in_=d2n[:, :],
                    in_offset=bass.IndirectOffsetOnAxis(ap=j2u[:, 0:1], axis=0))

                t1 = smpool.tile([P, D], F32, tag="t1")
                nc.gpsimd.tensor_sub(t1, g1, d1nc)
                t2 = smpool.tile([P, D], F32, tag="t2")
                nc.gpsimd.tensor_sub(t2, g2, d1nc)
                sc1 = smpool.tile([P, D], F32, tag="sc1")
                nc.gpsimd.tensor_mul(sc1, t1, t1)
                dist1 = smpool.tile([P, 1], F32, tag="dist1")
                nc.vector.reduce_sum(dist1, sc1, axis=mybir.AxisListType.X)
                sc2 = smpool.tile([P, D], F32, tag="sc2")
                nc.gpsimd.tensor_mul(sc2, t2, t2)
                dist2 = smpool.tile([P, 1], F32, tag="dist2")
                nc.vector.reduce_sum(dist2, sc2, axis=mybir.AxisListType.X)

                # ratio = sqrt(dist1/dist2), Newton-refined
                rec = smpool.tile([P, 1], F32, tag="rec")
                nc.vector.reciprocal(rec, dist2)
                q = smpool.tile([P, 1], F32, tag="q")
                nc.gpsimd.tensor_mul(q, dist1, rec)
                r0 = smpool.tile([P, 1], F32, tag="r0")
                nc.scalar.activation(r0, q, AF.Sqrt)
                rr = smpool.tile([P, 1], F32, tag="rr")
                nc.vector.reciprocal(rr, r0)
                t = smpool.tile([P, 1], F32, tag="t")
                nc.gpsimd.tensor_mul(t, q, rr)
                r1 = smpool.tile([P, 1], F32, tag="r1")
                nc.gpsimd.tensor_add(r1, r0, t)
                nc.gpsimd.tensor_scalar_mul(r1, r1, 0.5)

                maskf = smpool.tile([P, 1], F32, tag="maskf")
                nc.vector.tensor_scalar(maskf, r1, TH, scalar2=None, op0=OP.is_le)
                nc.gpsimd.tensor_mul(outr[:, rt:rt + 1], r1, maskf)
                nc.gpsimd.tensor_copy(outm[:, rt:rt + 1], maskf)

                jm = smpool.tile([P, 1], F32, tag="jm")
                nc.gpsimd.tensor_scalar_add(jm, j1, 1.0)
                nc.gpsimd.tensor_mul(jm, jm, maskf)
                nc.gpsimd.tensor_scalar_sub(outi[:, rt:rt + 1], jm, 1.0)

            # ---------------- outputs ---------------------------------------
            outi_i = cpool.tile([P, RT], I32, tag="outi_i")
            nc.gpsimd.tensor_copy(outi_i, outi)
            outm_u = cpool.tile([P, RT], U8, tag="outm_u")
            nc.gpsimd.tensor_copy(outm_u, outm)
            nc.sync.dma_start(out=o_ratio[:, :], in_=outr)
            nc.sync.dma_start(out=o_idx[:, :], in_=outi_i)
            nc.sync.dma_start(out=o_mask[:, :], in_=outm_u)

    nc.finalize()
    return nc


def _get_nc():
    if "nc" not in _CACHE:
        _CACHE["nc"] = build_nc()
    return _CACHE["nc"]


def _hilo(x):
    """bf16 two-term encoding (hi = bf16(x), lo = bf16(x - hi))."""
    hi = x.astype(ml_dtypes.bfloat16)
    lo = (x - hi.astype(np.float32)).astype(ml_dtypes.bfloat16)
    return hi, lo


def kernel(desc1, desc2, _trace=False, _tmpdir=None):
    desc1 = np.ascontiguousarray(np.asarray(desc1, dtype=np.float32))
    desc2 = np.ascontiguousarray(np.asarray(desc2, dtype=np.float32))
    assert desc1.shape == (B1, D) and desc2.shape == (B2, D)

    d2t = np.ascontiguousarray(desc2.T)
    d2hv, d2lv = _hilo(d2t)
    d2hv = np.ascontiguousarray(d2hv)
    d2lv = np.ascontiguousarray(d2lv)
    in_maps = []
    for c in range(NCORES):
        sl = slice(c * ROWS_PER_CORE, (c + 1) * ROWS_PER_CORE)
        d1n_c = np.ascontiguousarray(desc1[sl])
        d1t_c = np.ascontiguousarray(d1n_c.T)
        d1h_c, d1l_c = _hilo(d1t_c)
        in_maps.append({
            "d1n": d1n_c,
            "d1h": np.ascontiguousarray(d1h_c),
            "d1l": np.ascontiguousarray(d1l_c),
            "d2t": d2t, "d2h": d2hv, "d2l": d2lv, "d2n": desc2,
        })

    nc = _get_nc()
    res = run_bass_kernel_spmd(
        nc, in_maps, core_ids=list(range(NCORES)),
        trace=_trace, tmpdir=_tmpdir,
    )
    if _trace:
        _CACHE["last_result"] = res

    ratios, idxs, masks = [], [], []
    for c in range(NCORES):
        r = res.results[c]
        ratios.append(r["o_ratio"].T.reshape(-1))
        idxs.append(r["o_idx"].T.reshape(-1))
        masks.append(r["o_mask"].T.reshape(-1))
    ratio = np.concatenate(ratios).astype(np.float32)
    idx1 = np.concatenate(idxs).astype(np.int32)
    mask = np.concatenate(masks).astype(bool)

    match_dists = ratio[:, None]
    col0 = np.where(mask, np.arange(B1, dtype=np.int32), np.int32(-1))
    matches_idxs = np.stack([col0, idx1], axis=1).astype(np.int32)
    return match_dists, matches_idxs, mask


# revision 17
# speedup vs baseline: 1.2652x; 1.1464x over previous
"""DescriptorMatcher (SNN ratio-test) Trainium2 kernel.

Problem: desc1, desc2 (8192, 128) f32. For each row i of desc1: find the two
nearest neighbours in desc2 (L2), ratio = d1/d2, mask = ratio <= 0.8, emit
(ratio-or-0, [i, argmin]-or--1, mask).

Sharding: desc1 rows split across 8 cores (1024 rows each), desc2 replicated.
Host-side prep is layout/encoding only: slicing, transposes, and a bf16
hi/lo two-term encoding of the descriptors (hi = bf16(x), lo = bf16(x-hi));
all O(B1*B2) math runs on device.

Per-core algorithm:
  - Ranking surface s = d1.d2 - ||d2||^2/2 = -(dist^2 - ||d1||^2)/2 per
    (128 x 512) chunk: 3 bf16 matmul passes (hi*hi + hi*lo + lo*hi) plus a
    K=2 matmul folding -||d2||^2/2 (bf16 hi/lo rows, computed on device).
  - ScalarE copies PSUM (128 x 1024 double-chunks) into SBUF half-row tiles.
  - Per half (128 x 4096): VectorE max8 + find_index8.
  - Merge halves with a compare/select chain -> global j1, j2.
  - Exact refinement: indirect-DMA gather desc2[j1], desc2[j2]; recompute
    both squared distances exactly in fp32; ratio = sqrt(d1sq/d2sq)
    (ACT sqrt + one Newton step); mask = ratio <= 0.8; masked outputs.
Outputs per core: ratio (128,8) f32, idx (128,8) i32, mask (128,8) u8;
column rt = row-tile rt (rows rt*128 .. rt*128+127 of the core's slice).
"""

import numpy as np
import ml_dtypes

import concourse.bass as bass
import concourse.mybir as mybir
from concourse.bacc import Bacc
from concourse.tile import TileContext
from concourse.bass_utils import run_bass_kernel_spmd

P = 128
B1 = 8192
B2 = 8192
D = 128
NCORES = 8
ROWS_PER_CORE = B1 // NCORES       # 1024
RT = ROWS_PER_CORE // P            # 8 row-tiles per core
N = 512                            # matmul free-dim chunk
NCH = B2 // N                      # 16 chunks
HALF = B2 // 2                     # 4096
CH_H = NCH // 2                    # 8 chunks per half
TH = 0.8

F32 = mybir.dt.float32
BF16 = mybir.dt.bfloat16
I32 = mybir.dt.int32
U8 = mybir.dt.uint8
U32 = mybir.dt.uint32
AF = mybir.ActivationFunctionType
OP = mybir.AluOpType

_CACHE = {}


def build_nc():
    nc = Bacc()
    d1n = nc.dram_tensor("d1n", [ROWS_PER_CORE, D], F32, kind="ExternalInput")
    d1h = nc.dram_tensor("d1h", [D, ROWS_PER_CORE], BF16, kind="ExternalInput")
    d1l = nc.dram_tensor("d1l", [D, ROWS_PER_CORE], BF16, kind="ExternalInput")
    d2h = nc.dram_tensor("d2h", [D, B2], BF16, kind="ExternalInput")
    d2l = nc.dram_tensor("d2l", [D, B2], BF16, kind="ExternalInput")
    d2n = nc.dram_tensor("d2n", [B2, D], F32, kind="ExternalInput")
    o_ratio = nc.dram_tensor("o_ratio", [P, RT], F32, kind="ExternalOutput")
    o_idx = nc.dram_tensor("o_idx", [P, RT], I32, kind="ExternalOutput")
    o_mask = nc.dram_tensor("o_mask", [P, RT], U8, kind="ExternalOutput")

    with TileContext(nc) as tc:
        with tc.tile_pool(name="const", bufs=1) as cpool, \
             tc.tile_pool(name="load", bufs=3) as ldpool, \
             tc.tile_pool(name="setup", bufs=2) as supool, \
             tc.tile_pool(name="scan", bufs=4) as spool, \
             tc.tile_pool(name="small", bufs=3) as smpool, \
             tc.tile_pool(name="ps_set", bufs=2, space="PSUM") as psB, \
             tc.tile_pool(name="ps_mm", bufs=5, space="PSUM") as psA:

            # ---------------- persistent tiles ----------------
            d2hi = cpool.tile([P, B2], BF16, tag="d2hi")
            d2lo = cpool.tile([P, B2], BF16, tag="d2lo")
            cf = cpool.tile([2, B2], BF16, tag="cf")       # -d2sq/2: hi, lo rows
            d1hi = cpool.tile([P, ROWS_PER_CORE], BF16, tag="d1hi")
            d1lo = cpool.tile([P, ROWS_PER_CORE], BF16, tag="d1lo")
            mhalf = cpool.tile([P, P], F32, tag="mhalf")   # -0.5 weights
            ones2 = cpool.tile([2, P], BF16, tag="ones2")
            outr = cpool.tile([P, RT], F32, tag="outr")
            outi = cpool.tile([P, RT], F32, tag="outi")
            outm = cpool.tile([P, RT], F32, tag="outm")

            nc.gpsimd.memset(mhalf, -0.5)
            nc.gpsimd.memset(ones2, 1.0)
            nc.sync.dma_start(out=d1hi, in_=d1h[:, :])
            nc.sync.dma_start(out=d1lo, in_=d1l[:, :])

            mone1 = cpool.tile([1, P], BF16, tag="mone1")
            nc.gpsimd.memset(mone1, -1.0)
            # ---------------- setup: c rows + d2 split loads, per chunk ----
            for n in range(NCH):
                sl = bass.ts(n, N)
                nc.sync.dma_start(out=d2hi[:, sl], in_=d2h[:, sl])
                nc.sync.dma_start(out=d2lo[:, sl], in_=d2l[:, sl])
                d2c = ldpool.tile([P, N], F32, tag="d2c")
                nc.gpsimd.tensor_add(d2c, d2hi[:, sl], d2lo[:, sl])
                sq = supool.tile([P, N], F32, tag="sq")
                nc.scalar.activation(sq, d2c, AF.Square)
                cps = psB.tile([P, N], F32, tag="csq")
                nc.tensor.matmul(cps, mhalf, sq, start=True, stop=False)
                chi = supool.tile([P, N], BF16, tag="chi")
                nc.scalar.copy(chi, cps)
                # cps -= chi via K=1 matmul -> cps holds c - chi (the lo part)
                nc.tensor.matmul(cps, mone1, chi[0:1, :], start=False, stop=True)
                clo = supool.tile([P, N], BF16, tag="clo")
                nc.scalar.copy(clo, cps)
                nc.sync.dma_start(out=cf[0:1, sl], in_=chi[0:1, :])
                nc.sync.dma_start(out=cf[1:2, sl], in_=clo[0:1, :])

            # ---------------- steady: per row-tile --------------------------
            for rt in range(RT):
                rsl = bass.ts(rt, P)
                l_hi = d1hi[:, rsl]
                l_lo = d1lo[:, rsl]
                d1nc = ldpool.tile([P, D], F32, tag="d1nc")
                nc.sync.dma_start(out=d1nc, in_=d1n[rsl, :])

                CV = []   # per half: (128,8) top values
                IV = []   # per half: (128,8) u32 indices (half-local)
                for h in range(2):
                    s_half = spool.tile([P, HALF], F32, tag="s")
                    for qq in range(2):              # quarters of 4 chunks
                        pss = []
                        sls = []
                        for k4 in range(4):
                            n = h * CH_H + qq * 4 + k4
                            sls.append(bass.ts(n, N))
                            pss.append(psA.tile([P, N], F32, tag="mm", name=f"mmq_{rt}_{h}_{qq}_{k4}"))
                        for k4 in range(4):
                            nc.tensor.matmul(pss[k4], l_hi, d2hi[:, sls[k4]], start=True, stop=False)
                        for k4 in range(4):
                            nc.tensor.matmul(pss[k4], l_hi, d2lo[:, sls[k4]], start=False, stop=False)
                        for k4 in range(4):
                            nc.tensor.matmul(pss[k4], l_lo, d2hi[:, sls[k4]], start=False, stop=False)
                        for k4 in range(4):
                            nc.tensor.matmul(pss[k4], ones2, cf[:, sls[k4]], start=False, stop=True)
                        for k4 in range(4):
                            k = qq * 4 + k4
                            nc.scalar.copy(s_half[:, bass.ts(k, N)], pss[k4])
                    m8 = smpool.tile([P, 8], F32, tag="m8")
                    nc.vector.max(out=m8, in_=s_half)
                    i8 = smpool.tile([P, 8], U32, tag="i8")
                    nc.vector.max_index(out=i8, in_max=m8, in_values=s_half)
                    CV.append(m8)
                    IV.append(i8)

                # ---- merge halves: global (j1, j2) via select chain --------
                A0 = CV[0][:, 0:1]; A1 = CV[0][:, 1:2]
                B0 = CV[1][:, 0:1]; B1 = CV[1][:, 1:2]
                IA0 = smpool.tile([P, 1], F32, tag="IA0")
                nc.gpsimd.tensor_copy(IA0, IV[0][:, 0:1])
                IA1 = smpool.tile([P, 1], F32, tag="IA1")
                nc.gpsimd.tensor_copy(IA1, IV[0][:, 1:2])
                IB0 = smpool.tile([P, 1], F32, tag="IB0")
                nc.gpsimd.tensor_copy(IB0, IV[1][:, 0:1])
                nc.gpsimd.tensor_scalar_add(IB0, IB0, float(HALF))
                IB1 = smpool.tile([P, 1], F32, tag="IB1")
                nc.gpsimd.tensor_copy(IB1, IV[1][:, 1:2])
                nc.gpsimd.tensor_scalar_add(IB1, IB1, float(HALF))

                c0 = smpool.tile([P, 1], U8, tag="c0")   # 1 if half0 wins
                nc.vector.tensor_tensor(c0, A0, B0, op=OP.is_ge)
                c1 = smpool.tile([P, 1], U8, tag="c1")   # A1 vs B0
                nc.vector.tensor_tensor(c1, A1, B0, op=OP.is_ge)
                c2 = smpool.tile([P, 1], U8, tag="c2")   # A0 vs B1
                nc.vector.tensor_tensor(c2, A0, B1, op=OP.is_ge)

                # j1 = c0 ? IA0 : IB0
                j1 = smpool.tile([P, 1], F32, tag="j1")
                nc.vector.select(j1, c0, IA0, IB0)
                # j2 = c0 ? (c1 ? IA1 : IB0) : (c2 ? IA0 : IB1)
                j2a = smpool.tile([P, 1], F32, tag="j2a")
                nc.vector.select(j2a, c1, IA1, IB0)
                j2b = smpool.tile([P, 1], F32, tag="j2b")
                nc.vector.select(j2b, c2, IA0, IB1)
                j2 = smpool.tile([P, 1], F32, tag="j2")
                nc.vector.select(j2, c0, j2a, j2b)

                j1u = smpool.tile([P, 1], U32, tag="j1u")
                nc.gpsimd.tensor_copy(j1u, j1)
                j2u = smpool.tile([P, 1], U32, tag="j2u")
                nc.gpsimd.tensor_copy(j2u, j2)

                # ---- exact refinement ----
                g1 = smpool.tile([P, D], F32, tag="g1")
                nc.gpsimd.indirect_dma_start(
                    out=g1, out_offset=None, in_=d2n[:, :],
                    in_offset=bass.IndirectOffsetOnAxis(ap=j1u[:, 0:1], axis=0))
                g2 = smpool.tile([P, D], F32, tag="g2")
                nc.gpsimd.indirect_dma_start(
                    out=g2, out_offset=None, in_=d2n[:, :],
                    in_offset=bass.IndirectOffsetOnAxis(ap=j2u[:, 0:1], axis=0))

                t1 = smpool.tile([P, D], F32, tag="t1")
                nc.gpsimd.tensor_sub(t1, g1, d1nc)
                t2 = smpool.tile([P, D], F32, tag="t2")
                nc.gpsimd.tensor_sub(t2, g2, d1nc)
                sc1 = smpool.tile([P, D], F32, tag="sc1")
                nc.gpsimd.tensor_mul(sc1, t1, t1)
                dist1 = smpool.tile([P, 1], F32, tag="dist1")
                nc.vector.reduce_sum(dist1, sc1, axis=mybir.AxisListType.X)
                sc2 = smpool.tile([P, D], F32, tag="sc2")
                nc.gpsimd.tensor_mul(sc2, t2, t2)
                dist2 = smpool.tile([P, 1], F32, tag="dist2")
                nc.vector.reduce_sum(dist2, sc2, axis=mybir.AxisListType.X)

                # ratio = sqrt(dist1/dist2), Newton-refined
                rec = smpool.tile([P, 1], F32, tag="rec")
                nc.vector.reciprocal(rec, dist2)
                q = smpool.tile([P, 1], F32, tag="q")
                nc.gpsimd.tensor_mul(q, dist1, rec)
                r0 = smpool.tile([P, 1], F32, tag="r0")
                nc.scalar.activation(r0, q, AF.Sqrt)
                rr = smpool.tile([P, 1], F32, tag="rr")
                nc.vector.reciprocal(rr, r0)
                t = smpool.tile([P, 1], F32, tag="t")
                nc.gpsimd.tensor_mul(t, q, rr)
                r1 = smpool.tile([P, 1], F32, tag="r1")
                nc.gpsimd.tensor_add(r1, r0, t)
                nc.gpsimd.tensor_scalar_mul(r1, r1, 0.5)

                maskf = smpool.tile([P, 1], F32, tag="maskf")
                nc.vector.tensor_scalar(maskf, r1, TH, scalar2=None, op0=OP.is_le)
                nc.gpsimd.tensor_mul(outr[:, rt:rt + 1], r1, maskf)
                nc.gpsimd.tensor_copy(outm[:, rt:rt + 1], maskf)

                jm = smpool.tile([P, 1], F32, tag="jm")
                nc.gpsimd.tensor_scalar_add(jm, j1, 1.0)
                nc.gpsimd.tensor_mul(jm, jm, maskf)
                nc.gpsimd.tensor_scalar_sub(outi[:, rt:rt + 1], jm, 1.0)

            # ---------------- outputs ---------------------------------------
            outi_i = cpool.tile([P, RT], I32, tag="outi_i")
            nc.gpsimd.tensor_copy(outi_i, outi)
            outm_u = cpool.tile([P, RT], U8, tag="outm_u")
            nc.gpsimd.tensor_copy(outm_u, outm)
            nc.sync.dma_start(out=o_ratio[:, :], in_=outr)
            nc.sync.dma_start(out=o_idx[:, :], in_=outi_i)
            nc.sync.dma_start(out=o_mask[:, :], in_=outm_u)

    nc.finalize()
    return nc


def _get_nc():
    if "nc" not in _CACHE:
        _CACHE["nc"] = build_nc()
    return _CACHE["nc"]


def _hilo(x):
    """bf16 two-term encoding (hi = bf16(x), lo = bf16(x - hi))."""
    hi = x.astype(ml_dtypes.bfloat16)
    lo = (x - hi.astype(np.float32)).astype(ml_dtypes.bfloat16)
    return hi, lo


def kernel(desc1, desc2, _trace=False, _tmpdir=None):
    desc1 = np.ascontiguousarray(np.asarray(desc1, dtype=np.float32))
    desc2 = np.ascontiguousarray(np.asarray(desc2, dtype=np.float32))
    assert desc1.shape == (B1, D) and desc2.shape == (B2, D)

    d2t = np.ascontiguousarray(desc2.T)
    d2hv, d2lv = _hilo(d2t)
    del d2t
    d2hv = np.ascontiguousarray(d2hv)
    d2lv = np.ascontiguousarray(d2lv)
    in_maps = []
    for c in range(NCORES):
        sl = slice(c * ROWS_PER_CORE, (c + 1) * ROWS_PER_CORE)
        d1n_c = np.ascontiguousarray(desc1[sl])
        d1t_c = np.ascontiguousarray(d1n_c.T)
        d1h_c, d1l_c = _hilo(d1t_c)
        in_maps.append({
            "d1n": d1n_c,
            "d1h": np.ascontiguousarray(d1h_c),
            "d1l": np.ascontiguousarray(d1l_c),
            "d2h": d2hv, "d2l": d2lv, "d2n": desc2,
        })

    nc = _get_nc()
    res = run_bass_kernel_spmd(
        nc, in_maps, core_ids=list(range(NCORES)),
        trace=_trace, tmpdir=_tmpdir,
    )
    if _trace:
        _CACHE["last_result"] = res

    ratios, idxs, masks = [], [], []
    for c in range(NCORES):
        r = res.results[c]
        ratios.append(r["o_ratio"].T.reshape(-1))
        idxs.append(r["o_idx"].T.reshape(-1))
        masks.append(r["o_mask"].T.reshape(-1))
    ratio = np.concatenate(ratios).astype(np.float32)
    idx1 = np.concatenate(idxs).astype(np.int32)
    mask = np.concatenate(masks).astype(bool)

    match_dists = ratio[:, None]
    col0 = np.where(mask, np.arange(B1, dtype=np.int32), np.int32(-1))
    matches_idxs = np.stack([col0, idx1], axis=1).astype(np.int32)
    return match_dists, matches_idxs, mask
